# revision 31
# baseline (speedup 1.0000x reference)
"""MiMoV2 sparse attention (GQA + sliding window + sink) on 8 TRN2 cores.

Tensor-parallel over heads: core c owns q heads 4c..4c+3 and kv head c
(GQA groups align with cores); host sums the 8 partial o_proj outputs.

Per-core dataflow, software-pipelined so the PE never idles:
  KV phase: kT/vT projections for all 4 token tiles (bf16 weights/h),
    RoPE on k, v transposed to [tok, d] via PE.
  Slots n=0..5: Q-proj(n) + attention(n-1) + o_proj(n-2), with the three
    instruction streams round-robin interleaved at ~1us granularity so
    exp/activation latency hides under projection/o_proj matmuls.

Attention per (head, 512-query tile): S^T[k,q] = kT.T @ qT (f32r psum),
  w = exp(S^T) bf16 on Act; partial-visibility tiles multiplied by
  precomputed 0/1 masks (DVE); attnT += v.T @ w (PE, accumulating);
  wsum += w (DVE/Pool alternating); denom = ones.T @ wsum (single 512-row
  matmul per head/tile); attnT *= broadcast(1/(denom + exp(sink))).
o_proj in transposed layout: outT[oc, tok] += woT.T @ attnT, written
  bf16 to a packed DRAM tensor; host sums partials and transposes (free).

Softmax uses a constant (zero) max-shift: scores are bounded (|s| < ~10)
far below fp32/bf16 exp overflow, and softmax is shift-invariant; the
sink logit enters the denominator as exp(sink).
"""
import os
import numpy as np
import ml_dtypes

import concourse.bass as bass
import concourse.mybir as mybir
import concourse.tile as tile
from concourse import bacc
from concourse.bass_utils import run_bass_kernel_spmd
from concourse.masks import make_identity
from contextlib import ExitStack

F32 = mybir.dt.float32
F32R = mybir.dt.float32r
BF16 = mybir.dt.bfloat16

S = 2048
HID = 4096
NQ = 32
NKV = 8
D = 128
WINDOW = 1024
THETA = 1e6
CORES = 8
QH = NQ // CORES          # 4 q heads per core
DQ = QH * D               # 512
NT = S // 512             # 4 token tiles of 512
KS = S // 128             # 16 key subtiles of 128

# q/k stored bf16 post-RoPE: the walrus verifier requires matmul operand
# dtypes to match when either is f32/f32r, and bf16 stationaries load 2x
# faster into the PE array.
KT_DT = BF16

last_results = None       # set by kernel(); test.py reads exec_time_ns


def _schedule(positions):
    """Static attention schedule from the actual positions array.

    Returns (masks_np [128, P*512] bf16, sched[qt] = list of (ks, pidx))
    where pidx is None for fully-visible key subtiles.
    """
    pos = np.asarray(positions).astype(np.int64)
    vis = (pos[None, :] <= pos[:, None]) & (pos[:, None] - pos[None, :] < WINDOW)
    patterns = {}
    plist = []
    sched = []
    for qt in range(NT):
        row = []
        for ks in range(KS):
            sub = vis[qt * 512:(qt + 1) * 512, ks * 128:(ks + 1) * 128]
            if not sub.any():
                continue
            if sub.all():
                row.append((ks, None))
            else:
                pat = np.ascontiguousarray(sub.T).astype(np.float32)  # [128 k, 512 q]
                key = pat.tobytes()
                if key not in patterns:
                    patterns[key] = len(plist)
                    plist.append(pat)
                row.append((ks, patterns[key]))
        sched.append(row)
    if not plist:
        plist = [np.ones((128, 512), np.float32)]
    masks = np.concatenate(plist, axis=1).astype(ml_dtypes.bfloat16)  # [128, P*512]
    return masks, sched, len(plist)


def _build(sched, n_patterns):
    nc = bacc.Bacc("TRN2", target_bir_lowering=False)

    # Host-packed inputs: every load is a contiguous [128, n] slab.
    HB = nc.dram_tensor("hb", [128, NT * 4 * 8 * 512], BF16, kind="ExternalInput")
    WKV = nc.dram_tensor("wkv", [128, 2 * 32 * 128], BF16, kind="ExternalInput")
    WQa = nc.dram_tensor("wqa", [128, 32 * 256], BF16, kind="ExternalInput")
    WQb = nc.dram_tensor("wqb", [128, 32 * 256], BF16, kind="ExternalInput")
    WOT = nc.dram_tensor("wot", [128, QH * 32 * 128], BF16, kind="ExternalInput")
    Cos = nc.dram_tensor("cos", [128, S], F32, kind="ExternalInput")
    Sin = nc.dram_tensor("sin", [128, S], F32, kind="ExternalInput")
    Mk = nc.dram_tensor("mk", [128, n_patterns * 512], BF16, kind="ExternalInput")
    One = nc.dram_tensor("one", [128, 1], BF16, kind="ExternalInput")
    Esk = nc.dram_tensor("esk", [1, QH], F32, kind="ExternalInput")
    # packed partial output: cols = qt*16384 + o*512 + t  (o = hid/128 chunk)
    OutP = nc.dram_tensor("outp", [128, NT * 32 * 512], BF16, kind="ExternalOutput")

    Exp = mybir.ActivationFunctionType.Exp

    with tile.TileContext(nc) as tc, ExitStack() as top:
        persist = top.enter_context(tc.tile_pool(name="persist", bufs=1))
        ones = persist.tile([128, 1], BF16)
        nc.sync.dma_start(ones[:], One[:])
        esk = persist.tile([1, QH], F32)
        nc.sync.dma_start(esk[:], Esk[:])
        cos_sb = persist.tile([128, S], F32)
        sin_sb = persist.tile([128, S], F32)
        mk_sb = persist.tile([128, n_patterns * 512], BF16)
        wot_sb = persist.tile([128, QH * 32 * 128], BF16)
        wq_sb = [persist.tile([128, 32 * 256], BF16, tag=f"wq{i}", name=f"wq{i}")
                 for i in range(2)]
        wkv_sb = persist.tile([128, 2 * 32 * 128], BF16)
        qT = [[persist.tile([128, 512], BF16, tag=f"qT{m}_{n}", name=f"qT{m}_{n}")
               for n in range(NT)] for m in range(QH)]
        kT = [persist.tile([128, 512], KT_DT, tag=f"kT{n}", name=f"kT{n}") for n in range(NT)]
        v_sb = [persist.tile([128, 512], BF16, tag=f"v{n}", name=f"v{n}") for n in range(NT)]

        def rope(dst, ps, n):
            co = cos_sb[:, n * 512:(n + 1) * 512]
            si = sin_sb[:, n * 512:(n + 1) * 512]
            t2 = rtmp.tile([128, 512], F32, tag="t2", name="t2")
            nc.vector.tensor_mul(t2[0:64, :], ps[64:128, :], si[0:64, :])
            nc.vector.tensor_mul(t2[64:128, :], ps[0:64, :], si[64:128, :])
            tc_ = rtmp.tile([128, 512], F32, tag="tc", name="tc")
            nc.vector.tensor_mul(tc_[:], ps[:], co)
            nc.vector.tensor_add(dst, tc_[:], t2[:])

        # ------- Slots: proj(n) [kv + q passes] + attn(n-1) + o_proj(n-2)
        with ExitStack() as pq:
            hqp = pq.enter_context(tc.tile_pool(name="hqp", bufs=1))
            rtmp = pq.enter_context(tc.tile_pool(name="rtmpq", bufs=2))
            vtp = pq.enter_context(tc.tile_pool(name="vtp", bufs=2))
            wpool = pq.enter_context(tc.tile_pool(name="wpool", bufs=7))
            wspool = pq.enter_context(tc.tile_pool(name="wspool", bufs=2))
            dpool = pq.enter_context(tc.tile_pool(name="dpool", bufs=2))
            apool = pq.enter_context(tc.tile_pool(name="apool", bufs=1))
            obp = pq.enter_context(tc.tile_pool(name="obp", bufs=2))
            ps_q = pq.enter_context(tc.tile_pool(name="ps_q", bufs=1, space="PSUM"))
            ps_s = pq.enter_context(tc.tile_pool(name="ps_s", bufs=2, space="PSUM"))
            ps_a = pq.enter_context(tc.tile_pool(name="ps_a", bufs=2, space="PSUM"))
            ps_o = pq.enter_context(tc.tile_pool(name="ps_o", bufs=2, space="PSUM"))

            attnT = [[None] * QH for _ in range(NT)]

            # Slot 0 is DMA-bound and a single queue tops out well below the
            # HBM rate, so its loads are spread across three issue queues in
            # first-use order: sync carries h, scalar carries wkv + cos/sin,
            # gpsimd carries wq + mk. wot (first use: slot 2) is enqueued
            # behind the first attention broadcast on gpsimd so its transfer
            # cannot steal bandwidth from the slot-0 critical path.
            nc.gpsimd.dma_start(wq_sb[0][:], WQa[:])
            nc.gpsimd.dma_start(wq_sb[1][:], WQb[:])
            nc.gpsimd.dma_start(mk_sb[:], Mk[:])

            def proj_stream(n):
                hqs = []
                for q in range(4):
                    hq = hqp.tile([128, 4096], BF16, tag=f"hq{q}")
                    nc.sync.dma_start(hq[:], HB[:, (n * 4 + q) * 4096:(n * 4 + q + 1) * 4096])
                    if n == 0:
                        nc.scalar.dma_start(wkv_sb[:, q * 1024:(q + 1) * 1024],
                                            WKV[:, q * 1024:(q + 1) * 1024])
                        nc.scalar.dma_start(wkv_sb[:, 4096 + q * 1024:4096 + (q + 1) * 1024],
                                            WKV[:, 4096 + q * 1024:4096 + (q + 1) * 1024])
                        if q == 3:
                            nc.scalar.dma_start(cos_sb[:, 0:512], Cos[:, 0:512])
                            nc.scalar.dma_start(sin_sb[:, 0:512], Sin[:, 0:512])
                    elif q == 0 and n == 1:
                        nc.scalar.dma_start(cos_sb[:, 512:1536], Cos[:, 512:1536])
                        nc.scalar.dma_start(sin_sb[:, 512:1536], Sin[:, 512:1536])
                    elif q == 0 and n == 2:
                        nc.scalar.dma_start(cos_sb[:, 1536:], Cos[:, 1536:])
                        nc.scalar.dma_start(sin_sb[:, 1536:], Sin[:, 1536:])
                    hqs.append(hq)
                    yield
                # pass 1: k and v projections
                kvps = [ps_q.tile([128, 512], F32, tag=f"proj{j}", name=f"proj{j}")
                        for j in range(2)]
                for q in range(4):
                    for m in range(2):
                        for half in range(2):
                            for kk in range(half * 4, half * 4 + 4):
                                nc.tensor.matmul(
                                    kvps[m][:],
                                    wkv_sb[:, m * 4096 + (q * 8 + kk) * 128:
                                           m * 4096 + (q * 8 + kk + 1) * 128],
                                    hqs[q][:, kk * 512:(kk + 1) * 512],
                                    start=(q == 0 and kk == 0), stop=(q == 3 and kk == 7))
                            yield
                rope(kT[n][:], kvps[0][:], n)
                yield
                vt = vtp.tile([128, 512], BF16, tag="vt", name=f"vt{n}")
                nc.scalar.copy(vt[:], kvps[1][:])
                for t in range(4):
                    nc.sync.dma_start_transpose(v_sb[n][:, t * 128:(t + 1) * 128],
                                                vt[:, t * 128:(t + 1) * 128])
                yield
                # passes 2/3: q heads in pairs, reusing the same two banks
                for mp in range(2):
                    qps = [ps_q.tile([128, 512], F32, tag=f"proj{j}", name=f"proj{j}")
                           for j in range(2)]
                    for q in range(4):
                        for j in range(2):
                            for half in range(2):
                                for kk in range(half * 4, half * 4 + 4):
                                    nc.tensor.matmul(
                                        qps[j][:],
                                        wq_sb[mp][:, (q * 8 + kk) * 256 + j * 128:
                                                (q * 8 + kk) * 256 + (j + 1) * 128],
                                        hqs[q][:, kk * 512:(kk + 1) * 512],
                                        start=(q == 0 and kk == 0),
                                        stop=(q == 3 and kk == 7))
                                yield
                    for j in range(2):
                        rope(qT[mp * 2 + j][n][:], qps[j][:], n)
                        yield

            def b_stream(qt):
                row = sched[qt]
                last = len(row) - 1

                def a_mm(a_ps, i, ks, w):
                    nc.tensor.matmul(
                        a_ps[:], v_sb[ks // 4][:, (ks % 4) * 128:(ks % 4 + 1) * 128],
                        w[:], start=(i == 0), stop=(i == last))

                for hd in range(QH):
                    a_ps = ps_a.tile([128, 512], F32, tag="a")
                    wsum = wspool.tile([128, 512], BF16, tag="ws")
                    pend = None  # (i, ks, w): AV matmul deferred one chunk
                    for i, (ks, pidx) in enumerate(row):
                        s_ps = ps_s.tile([128, 512], F32, tag="s")
                        nc.tensor.matmul(
                            s_ps[:], kT[ks // 4][:, (ks % 4) * 128:(ks % 4 + 1) * 128],
                            qT[hd][qt][:], start=True, stop=True)
                        if pend is not None:
                            a_mm(a_ps, *pend)
                        w = wpool.tile([128, 512], BF16, tag="w")
                        nc.scalar.activation(w[:], s_ps[:], Exp)
                        if pidx is not None:
                            nc.vector.tensor_mul(
                                w[:], w[:], mk_sb[:, pidx * 512:(pidx + 1) * 512])
                        if i == 0:
                            nc.vector.tensor_copy(wsum[:], w[:])
                        else:
                            nc.vector.tensor_add(wsum[:], wsum[:], w[:])
                        pend = (i, ks, w)
                        yield
                    a_mm(a_ps, *pend)
                    d_ps = ps_s.tile([128, 512], F32, tag="s")
                    nc.tensor.matmul(d_ps[0:1, :], ones[:], wsum[:], start=True, stop=True)
                    den = dpool.tile([1, 512], F32, tag="den")
                    nc.vector.tensor_scalar_add(den[:], d_ps[0:1, :], esk[0:1, hd:hd + 1])
                    rec = dpool.tile([1, 512], F32, tag="rec")
                    nc.vector.reciprocal_approx_fast(rec[:], den[:])
                    rbc = dpool.tile([128, 512], F32, tag="rbc")
                    nc.gpsimd.partition_broadcast(rbc[:], rec[:])
                    if qt == 0 and hd == 0:
                        # deferred bulk prefetch: queued after the broadcast
                        # above, so it starts once slot 1 is underway
                        nc.gpsimd.dma_start(wot_sb[:], WOT[:])
                    at = apool.tile([128, 512], BF16, tag=f"at{hd}_{qt % 2}")
                    nc.vector.tensor_mul(at[:], a_ps[:], rbc[:])
                    attnT[qt][hd] = at
                    yield

            def c_stream(qt):
                # the last tile runs with no other stream to hide the PSUM
                # drain, so borrow the idle proj banks for 4-deep pipelining
                deep = qt == NT - 1
                for og in range(8):
                    ob = obp.tile([128, 2048], BF16, tag="ob")
                    for c in range(4):
                        o = og * 4 + c
                        if deep and c % 2:
                            o_ps = ps_q.tile([128, 512], F32, tag=f"proj{og % 2}",
                                             name="o_ps")
                        else:
                            o_ps = ps_o.tile([128, 512], F32, tag="o")
                        for hd in range(QH):
                            nc.tensor.matmul(
                                o_ps[:],
                                wot_sb[:, (hd * 32 + o) * 128:(hd * 32 + o + 1) * 128],
                                attnT[qt][hd][:],
                                start=(hd == 0), stop=(hd == QH - 1))
                        if c % 2:
                            nc.scalar.copy(ob[:, c * 512:(c + 1) * 512], o_ps[:])
                        else:
                            nc.vector.tensor_copy(ob[:, c * 512:(c + 1) * 512], o_ps[:])
                        yield
                    nc.gpsimd.dma_start(
                        OutP[:, qt * 16384 + og * 2048:qt * 16384 + (og + 1) * 2048], ob[:])
                    yield
                    if not deep:
                        yield  # stretch to pace the longer b_stream rows

            for slot in range(NT + 2):
                streams = []
                if slot < NT:
                    streams.append(proj_stream(slot))
                if 0 <= slot - 1 < NT:
                    streams.append(b_stream(slot - 1))
                if 0 <= slot - 2 < NT:
                    streams.append(c_stream(slot - 2))
                while streams:
                    alive = []
                    for st in streams:
                        try:
                            next(st)
                            alive.append(st)
                        except StopIteration:
                            pass
                    streams = alive

    nc.compile()
    return nc


def kernel(hidden_states, positions, wq, wk, wv, wo, sink):
    global last_results
    h = np.asarray(hidden_states, np.float32)
    pos = np.asarray(positions)
    wq = np.asarray(wq, np.float32)
    wk = np.asarray(wk, np.float32)
    wv = np.asarray(wv, np.float32)
    wo = np.asarray(wo, np.float32)
    sink = np.asarray(sink, np.float32)

    masks, sched, n_pat = _schedule(pos)
    nc = _build(sched, n_pat)

    # h packed: [p, (n, q, kt, t)] = h[n*512+t, (q*8+kt)*128+p]
    hp = np.ascontiguousarray(
        h.reshape(NT, 512, 4, 8, 128).transpose(4, 0, 2, 3, 1).reshape(128, -1)
    ).astype(ml_dtypes.bfloat16)

    # RoPE tables (neox half-split), rows duplicated for both halves
    inv_freq = 1.0 / (THETA ** (np.arange(0, D, 2, dtype=np.float64) / D))
    freqs = pos.astype(np.float64)[:, None] * inv_freq[None, :]       # [S, 64]
    cos = np.cos(freqs).astype(np.float32).T                          # [64, S]
    sin = np.sin(freqs).astype(np.float32).T
    cos_full = np.ascontiguousarray(np.concatenate([cos, cos], axis=0))
    sin_sign = np.ascontiguousarray(np.concatenate([-sin, sin], axis=0))

    scale = np.float32(D ** -0.5)
    ones = np.ones((128, 1), np.float32)
    esink = np.exp(sink.astype(np.float64)).astype(np.float32)

    in_maps = []
    for c in range(CORES):
        wqc = (wq[:, c * DQ:(c + 1) * DQ] * scale)                    # [HID, 512]
        # [p, kt, m*128+i] = wqc[kt*128+p, :], split into head pairs
        wqr = wqc.reshape(32, 128, 512).transpose(1, 0, 2)
        wqa = np.ascontiguousarray(wqr[:, :, 0:256].reshape(128, -1)).astype(ml_dtypes.bfloat16)
        wqb = np.ascontiguousarray(wqr[:, :, 256:512].reshape(128, -1)).astype(ml_dtypes.bfloat16)
        wkc = wk[:, c * D:(c + 1) * D].reshape(32, 128, 128).transpose(1, 0, 2)
        wvc = wv[:, c * D:(c + 1) * D].reshape(32, 128, 128).transpose(1, 0, 2)
        wkvp = np.ascontiguousarray(
            np.concatenate([wkc.reshape(128, -1), wvc.reshape(128, -1)], axis=1)
        ).astype(ml_dtypes.bfloat16)
        woc = wo[c * DQ:(c + 1) * DQ, :]                              # [512, HID]
        # [p, hd, o, i] = woc[hd*128+p, o*128+i]
        wotp = np.ascontiguousarray(
            woc.reshape(QH, 128, 32, 128).transpose(1, 0, 2, 3).reshape(128, -1)
        ).astype(ml_dtypes.bfloat16)
        in_maps.append({
            "hb": hp,
            "wqa": wqa,
            "wqb": wqb,
            "wkv": wkvp,
            "wot": wotp,
            "cos": cos_full,
            "sin": sin_sign,
            "mk": masks,
            "one": ones.astype(ml_dtypes.bfloat16),
            "esk": np.ascontiguousarray(esink[None, c * QH:(c + 1) * QH]),
        })

    trace = bool(int(os.environ.get("KERNEL_TRACE", "0")))
    res = run_bass_kernel_spmd(nc, in_maps, core_ids=list(range(CORES)), trace=trace)
    last_results = res
    acc = np.zeros((128, NT * 32 * 512), np.float64)
    for r in res.results:
        acc += r["outp"].astype(np.float64)
    # [p, qt, o, t] -> out[qt*512+t, o*128+p]
    out = acc.reshape(128, NT, 32, 512).transpose(1, 3, 2, 0).reshape(S, HID)
    return out.astype(np.float32)


# revision 32
# speedup vs baseline: 1.1746x; 1.1746x over previous
"""MiMoV2 sparse attention (GQA + sliding window + sink) on 8 TRN2 cores.

Tensor-parallel over heads: core c owns q heads 4c..4c+3 and kv head c
(GQA groups align with cores); host sums the 8 partial o_proj outputs.

Per-core dataflow, software-pipelined so the PE never idles:
  KV phase: kT/vT projections for all 4 token tiles (bf16 weights/h),
    RoPE on k, v transposed to [tok, d] via PE.
  Slots n=0..5: Q-proj(n) + attention(n-1) + o_proj(n-2), with the three
    instruction streams round-robin interleaved at ~1us granularity so
    exp/activation latency hides under projection/o_proj matmuls.

Attention per (head, 512-query tile): S^T[k,q] = kT.T @ qT (f32r psum),
  w = exp(S^T) bf16 on Act; partial-visibility tiles multiplied by
  precomputed 0/1 masks (DVE); attnT += v.T @ w (PE, accumulating);
  wsum += w (DVE/Pool alternating); denom = ones.T @ wsum (single 512-row
  matmul per head/tile); attnT *= broadcast(1/(denom + exp(sink))).
o_proj in transposed layout: outT[oc, tok] += woT.T @ attnT, written
  bf16 to a packed DRAM tensor; host sums partials and transposes (free).

Softmax uses a constant (zero) max-shift: scores are bounded (|s| < ~10)
far below fp32/bf16 exp overflow, and softmax is shift-invariant; the
sink logit enters the denominator as exp(sink).
"""
import os
import numpy as np
import ml_dtypes

import concourse.bass as bass
import concourse.mybir as mybir
import concourse.tile as tile
from concourse import bacc
from concourse.bass_utils import run_bass_kernel_spmd
from concourse.masks import make_identity
from contextlib import ExitStack

F32 = mybir.dt.float32
F32R = mybir.dt.float32r
BF16 = mybir.dt.bfloat16

S = 2048
HID = 4096
NQ = 32
NKV = 8
D = 128
WINDOW = 1024
THETA = 1e6
CORES = 8
QH = NQ // CORES          # 4 q heads per core
DQ = QH * D               # 512
NT = S // 512             # 4 token tiles of 512
KS = S // 128             # 16 key subtiles of 128

# q/k stored bf16 post-RoPE: the walrus verifier requires matmul operand
# dtypes to match when either is f32/f32r, and bf16 stationaries load 2x
# faster into the PE array.
KT_DT = BF16

last_results = None       # set by kernel(); test.py reads exec_time_ns


def _schedule(positions):
    """Static attention schedule from the actual positions array.

    Returns (masks_np [128, P*512] bf16, sched[qt] = list of (ks, pidx))
    where pidx is None for fully-visible key subtiles.
    """
    pos = np.asarray(positions).astype(np.int64)
    vis = (pos[None, :] <= pos[:, None]) & (pos[:, None] - pos[None, :] < WINDOW)
    patterns = {}
    plist = []
    sched = []
    for qt in range(NT):
        row = []
        for ks in range(KS):
            sub = vis[qt * 512:(qt + 1) * 512, ks * 128:(ks + 1) * 128]
            if not sub.any():
                continue
            if sub.all():
                row.append((ks, None))
            else:
                pat = np.ascontiguousarray(sub.T).astype(np.float32)  # [128 k, 512 q]
                key = pat.tobytes()
                if key not in patterns:
                    patterns[key] = len(plist)
                    plist.append(pat)
                row.append((ks, patterns[key]))
        sched.append(row)
    if not plist:
        plist = [np.ones((128, 512), np.float32)]
    masks = np.concatenate(plist, axis=1).astype(ml_dtypes.bfloat16)  # [128, P*512]
    return masks, sched, len(plist)


def _build(sched, n_patterns):
    nc = bacc.Bacc("TRN2", target_bir_lowering=False)

    # Host-packed inputs: every load is a contiguous [128, n] slab.
    HB = nc.dram_tensor("hb", [128, NT * 4 * 8 * 512], BF16, kind="ExternalInput")
    WKV = nc.dram_tensor("wkv", [128, 2 * 32 * 128], BF16, kind="ExternalInput")
    WQa = nc.dram_tensor("wqa", [128, 32 * 256], BF16, kind="ExternalInput")
    WQb = nc.dram_tensor("wqb", [128, 32 * 256], BF16, kind="ExternalInput")
    WOT = nc.dram_tensor("wot", [128, QH * 32 * 128], BF16, kind="ExternalInput")
    Cos = nc.dram_tensor("cos", [128, S], F32, kind="ExternalInput")
    Sin = nc.dram_tensor("sin", [128, S], F32, kind="ExternalInput")
    Mk = nc.dram_tensor("mk", [128, n_patterns * 512], BF16, kind="ExternalInput")
    One = nc.dram_tensor("one", [128, 1], BF16, kind="ExternalInput")
    Esk = nc.dram_tensor("esk", [1, QH], F32, kind="ExternalInput")
    # packed partial output: cols = qt*16384 + o*512 + t  (o = hid/128 chunk)
    OutP = nc.dram_tensor("outp", [128, NT * 32 * 512], BF16, kind="ExternalOutput")

    Exp = mybir.ActivationFunctionType.Exp

    with tile.TileContext(nc) as tc, ExitStack() as top:
        persist = top.enter_context(tc.tile_pool(name="persist", bufs=1))
        ones = persist.tile([128, 1], BF16)
        nc.sync.dma_start(ones[:], One[:])
        esk = persist.tile([1, QH], F32)
        nc.sync.dma_start(esk[:], Esk[:])
        cos_sb = persist.tile([128, S], F32)
        sin_sb = persist.tile([128, S], F32)
        mk_sb = persist.tile([128, n_patterns * 512], BF16)
        wot_sb = persist.tile([128, QH * 32 * 128], BF16)
        wq_sb = [persist.tile([128, 32 * 256], BF16, tag=f"wq{i}", name=f"wq{i}")
                 for i in range(2)]
        wkv_sb = persist.tile([128, 2 * 32 * 128], BF16)
        qT = [[persist.tile([128, 512], BF16, tag=f"qT{m}_{n}", name=f"qT{m}_{n}")
               for n in range(NT)] for m in range(QH)]
        kT = [persist.tile([128, 512], KT_DT, tag=f"kT{n}", name=f"kT{n}") for n in range(NT)]
        v_sb = [persist.tile([128, 512], BF16, tag=f"v{n}", name=f"v{n}") for n in range(NT)]

        def rope(dst, ps, n):
            co = cos_sb[:, n * 512:(n + 1) * 512]
            si = sin_sb[:, n * 512:(n + 1) * 512]
            t2 = rtmp.tile([128, 512], F32, tag="t2", name="t2")
            nc.vector.tensor_mul(t2[0:64, :], ps[64:128, :], si[0:64, :])
            nc.vector.tensor_mul(t2[64:128, :], ps[0:64, :], si[64:128, :])
            tc_ = rtmp.tile([128, 512], F32, tag="tc", name="tc")
            nc.vector.tensor_mul(tc_[:], ps[:], co)
            nc.vector.tensor_add(dst, tc_[:], t2[:])

        # ------- Slots: proj(n) [kv + q passes] + attn(n-1) + o_proj(n-2)
        with ExitStack() as pq:
            hqp = pq.enter_context(tc.tile_pool(name="hqp", bufs=1))
            rtmp = pq.enter_context(tc.tile_pool(name="rtmpq", bufs=2))
            vtp = pq.enter_context(tc.tile_pool(name="vtp", bufs=2))
            wpool = pq.enter_context(tc.tile_pool(name="wpool", bufs=7))
            wspool = pq.enter_context(tc.tile_pool(name="wspool", bufs=2))
            dpool = pq.enter_context(tc.tile_pool(name="dpool", bufs=2))
            apool = pq.enter_context(tc.tile_pool(name="apool", bufs=1))
            obp = pq.enter_context(tc.tile_pool(name="obp", bufs=2))
            ps_q = pq.enter_context(tc.tile_pool(name="ps_q", bufs=1, space="PSUM"))
            ps_s = pq.enter_context(tc.tile_pool(name="ps_s", bufs=2, space="PSUM"))
            ps_a = pq.enter_context(tc.tile_pool(name="ps_a", bufs=2, space="PSUM"))
            ps_o = pq.enter_context(tc.tile_pool(name="ps_o", bufs=2, space="PSUM"))

            attnT = [[None] * QH for _ in range(NT)]

            # Slot 0 is DMA-bound and a single queue tops out well below the
            # HBM rate, so its loads are spread across three issue queues in
            # first-use order: sync carries h, scalar carries wkv + cos/sin,
            # gpsimd carries wq + mk. wot (first use: slot 2) is enqueued
            # behind the first attention broadcast on gpsimd so its transfer
            # cannot steal bandwidth from the slot-0 critical path.
            nc.gpsimd.dma_start(wq_sb[0][:], WQa[:])
            nc.gpsimd.dma_start(wq_sb[1][:], WQb[:])
            nc.gpsimd.dma_start(mk_sb[:], Mk[:])

            def proj_stream(n):
                hqs = []
                for q in range(4):
                    hq = hqp.tile([128, 4096], BF16, tag=f"hq{q}")
                    nc.sync.dma_start(hq[:], HB[:, (n * 4 + q) * 4096:(n * 4 + q + 1) * 4096])
                    if n == 0:
                        nc.sync.dma_start(wkv_sb[:, q * 1024:(q + 1) * 1024],
                                          WKV[:, q * 1024:(q + 1) * 1024])
                        nc.sync.dma_start(wkv_sb[:, 4096 + q * 1024:4096 + (q + 1) * 1024],
                                          WKV[:, 4096 + q * 1024:4096 + (q + 1) * 1024])
                        if q == 3:
                            nc.sync.dma_start(cos_sb[:, 0:512], Cos[:, 0:512])
                            nc.sync.dma_start(sin_sb[:, 0:512], Sin[:, 0:512])
                    elif q == 0 and n == 1:
                        nc.sync.dma_start(cos_sb[:, 512:1536], Cos[:, 512:1536])
                        nc.sync.dma_start(sin_sb[:, 512:1536], Sin[:, 512:1536])
                    elif q == 0 and n == 2:
                        nc.sync.dma_start(cos_sb[:, 1536:], Cos[:, 1536:])
                        nc.sync.dma_start(sin_sb[:, 1536:], Sin[:, 1536:])
                    hqs.append(hq)
                    yield
                # pass 1: k and v projections
                kvps = [ps_q.tile([128, 512], F32, tag=f"proj{j}", name=f"proj{j}")
                        for j in range(2)]
                for q in range(4):
                    for m in range(2):
                        for half in range(2):
                            for kk in range(half * 4, half * 4 + 4):
                                nc.tensor.matmul(
                                    kvps[m][:],
                                    wkv_sb[:, m * 4096 + (q * 8 + kk) * 128:
                                           m * 4096 + (q * 8 + kk + 1) * 128],
                                    hqs[q][:, kk * 512:(kk + 1) * 512],
                                    start=(q == 0 and kk == 0), stop=(q == 3 and kk == 7))
                            yield
                rope(kT[n][:], kvps[0][:], n)
                yield
                vt = vtp.tile([128, 512], BF16, tag="vt", name=f"vt{n}")
                nc.scalar.copy(vt[:], kvps[1][:])
                for t in range(4):
                    nc.sync.dma_start_transpose(v_sb[n][:, t * 128:(t + 1) * 128],
                                                vt[:, t * 128:(t + 1) * 128])
                yield
                # passes 2/3: q heads in pairs, reusing the same two banks
                for mp in range(2):
                    qps = [ps_q.tile([128, 512], F32, tag=f"proj{j}", name=f"proj{j}")
                           for j in range(2)]
                    for q in range(4):
                        for j in range(2):
                            for half in range(2):
                                for kk in range(half * 4, half * 4 + 4):
                                    nc.tensor.matmul(
                                        qps[j][:],
                                        wq_sb[mp][:, (q * 8 + kk) * 256 + j * 128:
                                                (q * 8 + kk) * 256 + (j + 1) * 128],
                                        hqs[q][:, kk * 512:(kk + 1) * 512],
                                        start=(q == 0 and kk == 0),
                                        stop=(q == 3 and kk == 7))
                                yield
                    for j in range(2):
                        rope(qT[mp * 2 + j][n][:], qps[j][:], n)
                        yield

            def b_stream(qt):
                row = sched[qt]
                last = len(row) - 1

                def a_mm(a_ps, i, ks, w):
                    nc.tensor.matmul(
                        a_ps[:], v_sb[ks // 4][:, (ks % 4) * 128:(ks % 4 + 1) * 128],
                        w[:], start=(i == 0), stop=(i == last))

                for hd in range(QH):
                    a_ps = ps_a.tile([128, 512], F32, tag="a")
                    wsum = wspool.tile([128, 512], BF16, tag="ws")
                    pend = None  # (i, ks, w): AV matmul deferred one chunk
                    for i, (ks, pidx) in enumerate(row):
                        s_ps = ps_s.tile([128, 512], F32, tag="s")
                        nc.tensor.matmul(
                            s_ps[:], kT[ks // 4][:, (ks % 4) * 128:(ks % 4 + 1) * 128],
                            qT[hd][qt][:], start=True, stop=True)
                        if pend is not None:
                            a_mm(a_ps, *pend)
                        w = wpool.tile([128, 512], BF16, tag="w")
                        nc.scalar.activation(w[:], s_ps[:], Exp)
                        if pidx is not None:
                            nc.vector.tensor_mul(
                                w[:], w[:], mk_sb[:, pidx * 512:(pidx + 1) * 512])
                        if i == 0:
                            nc.vector.tensor_copy(wsum[:], w[:])
                        else:
                            nc.vector.tensor_add(wsum[:], wsum[:], w[:])
                        pend = (i, ks, w)
                        yield
                    a_mm(a_ps, *pend)
                    d_ps = ps_s.tile([128, 512], F32, tag="s")
                    nc.tensor.matmul(d_ps[0:1, :], ones[:], wsum[:], start=True, stop=True)
                    den = dpool.tile([1, 512], F32, tag="den")
                    nc.vector.tensor_scalar_add(den[:], d_ps[0:1, :], esk[0:1, hd:hd + 1])
                    rec = dpool.tile([1, 512], F32, tag="rec")
                    nc.vector.reciprocal_approx_fast(rec[:], den[:])
                    rbc = dpool.tile([128, 512], F32, tag="rbc")
                    nc.gpsimd.partition_broadcast(rbc[:], rec[:])
                    if qt == 0 and hd == 0:
                        # deferred bulk prefetch: queued after the broadcast
                        # above, so it starts once slot 1 is underway
                        nc.gpsimd.dma_start(wot_sb[:], WOT[:])
                    at = apool.tile([128, 512], BF16, tag=f"at{hd}_{qt % 2}")
                    nc.vector.tensor_mul(at[:], a_ps[:], rbc[:])
                    attnT[qt][hd] = at
                    yield

            def c_stream(qt):
                # the last tile runs with no other stream to hide the PSUM
                # drain, so borrow the idle proj banks for 4-deep pipelining
                deep = qt == NT - 1
                for og in range(8):
                    ob = obp.tile([128, 2048], BF16, tag="ob")
                    for c in range(4):
                        o = og * 4 + c
                        if deep and c % 2:
                            o_ps = ps_q.tile([128, 512], F32, tag=f"proj{og % 2}",
                                             name="o_ps")
                        else:
                            o_ps = ps_o.tile([128, 512], F32, tag="o")
                        for hd in range(QH):
                            nc.tensor.matmul(
                                o_ps[:],
                                wot_sb[:, (hd * 32 + o) * 128:(hd * 32 + o + 1) * 128],
                                attnT[qt][hd][:],
                                start=(hd == 0), stop=(hd == QH - 1))
                        if c % 2:
                            nc.scalar.copy(ob[:, c * 512:(c + 1) * 512], o_ps[:])
                        else:
                            nc.vector.tensor_copy(ob[:, c * 512:(c + 1) * 512], o_ps[:])
                        yield
                    nc.gpsimd.dma_start(
                        OutP[:, qt * 16384 + og * 2048:qt * 16384 + (og + 1) * 2048], ob[:])
                    yield
                    if not deep:
                        yield  # stretch to pace the longer b_stream rows

            for slot in range(NT + 2):
                streams = []
                if slot < NT:
                    streams.append(proj_stream(slot))
                if 0 <= slot - 1 < NT:
                    streams.append(b_stream(slot - 1))
                if 0 <= slot - 2 < NT:
                    streams.append(c_stream(slot - 2))
                while streams:
                    alive = []
                    for st in streams:
                        try:
                            next(st)
                            alive.append(st)
                        except StopIteration:
                            pass
                    streams = alive

    nc.compile()
    return nc


def kernel(hidden_states, positions, wq, wk, wv, wo, sink):
    global last_results
    h = np.asarray(hidden_states, np.float32)
    pos = np.asarray(positions)
    wq = np.asarray(wq, np.float32)
    wk = np.asarray(wk, np.float32)
    wv = np.asarray(wv, np.float32)
    wo = np.asarray(wo, np.float32)
    sink = np.asarray(sink, np.float32)

    masks, sched, n_pat = _schedule(pos)
    nc = _build(sched, n_pat)

    # h packed: [p, (n, q, kt, t)] = h[n*512+t, (q*8+kt)*128+p]
    hp = np.ascontiguousarray(
        h.reshape(NT, 512, 4, 8, 128).transpose(4, 0, 2, 3, 1).reshape(128, -1)
    ).astype(ml_dtypes.bfloat16)

    # RoPE tables (neox half-split), rows duplicated for both halves
    inv_freq = 1.0 / (THETA ** (np.arange(0, D, 2, dtype=np.float64) / D))
    freqs = pos.astype(np.float64)[:, None] * inv_freq[None, :]       # [S, 64]
    cos = np.cos(freqs).astype(np.float32).T                          # [64, S]
    sin = np.sin(freqs).astype(np.float32).T
    cos_full = np.ascontiguousarray(np.concatenate([cos, cos], axis=0))
    sin_sign = np.ascontiguousarray(np.concatenate([-sin, sin], axis=0))

    scale = np.float32(D ** -0.5)
    ones = np.ones((128, 1), np.float32)
    esink = np.exp(sink.astype(np.float64)).astype(np.float32)

    in_maps = []
    for c in range(CORES):
        wqc = (wq[:, c * DQ:(c + 1) * DQ] * scale)                    # [HID, 512]
        # [p, kt, m*128+i] = wqc[kt*128+p, :], split into head pairs
        wqr = wqc.reshape(32, 128, 512).transpose(1, 0, 2)
        wqa = np.ascontiguousarray(wqr[:, :, 0:256].reshape(128, -1)).astype(ml_dtypes.bfloat16)
        wqb = np.ascontiguousarray(wqr[:, :, 256:512].reshape(128, -1)).astype(ml_dtypes.bfloat16)
        wkc = wk[:, c * D:(c + 1) * D].reshape(32, 128, 128).transpose(1, 0, 2)
        wvc = wv[:, c * D:(c + 1) * D].reshape(32, 128, 128).transpose(1, 0, 2)
        wkvp = np.ascontiguousarray(
            np.concatenate([wkc.reshape(128, -1), wvc.reshape(128, -1)], axis=1)
        ).astype(ml_dtypes.bfloat16)
        woc = wo[c * DQ:(c + 1) * DQ, :]                              # [512, HID]
        # [p, hd, o, i] = woc[hd*128+p, o*128+i]
        wotp = np.ascontiguousarray(
            woc.reshape(QH, 128, 32, 128).transpose(1, 0, 2, 3).reshape(128, -1)
        ).astype(ml_dtypes.bfloat16)
        in_maps.append({
            "hb": hp,
            "wqa": wqa,
            "wqb": wqb,
            "wkv": wkvp,
            "wot": wotp,
            "cos": cos_full,
            "sin": sin_sign,
            "mk": masks,
            "one": ones.astype(ml_dtypes.bfloat16),
            "esk": np.ascontiguousarray(esink[None, c * QH:(c + 1) * QH]),
        })

    trace = bool(int(os.environ.get("KERNEL_TRACE", "0")))
    res = run_bass_kernel_spmd(nc, in_maps, core_ids=list(range(CORES)), trace=trace)
    last_results = res
    acc = np.zeros((128, NT * 32 * 512), np.float64)
    for r in res.results:
        acc += r["outp"].astype(np.float64)
    # [p, qt, o, t] -> out[qt*512+t, o*128+p]
    out = acc.reshape(128, NT, 32, 512).transpose(1, 3, 2, 0).reshape(S, HID)
    return out.astype(np.float32)


# revision 33
# speedup vs baseline: 1.2113x; 1.0313x over previous
"""MiMoV2 sparse attention (GQA + sliding window + sink) on 8 TRN2 cores.

Tensor-parallel over heads: core c owns q heads 4c..4c+3 and kv head c
(GQA groups align with cores); host sums the 8 partial o_proj outputs.

Per-core dataflow, software-pipelined so the PE never idles:
  KV phase: kT/vT projections for all 4 token tiles (bf16 weights/h),
    RoPE on k, v transposed to [tok, d] via PE.
  Slots n=0..5: Q-proj(n) + attention(n-1) + o_proj(n-2), with the three
    instruction streams round-robin interleaved at ~1us granularity so
    exp/activation latency hides under projection/o_proj matmuls.

Attention per (head, 512-query tile): S^T[k,q] = kT.T @ qT (f32r psum),
  w = exp(S^T) bf16 on Act; partial-visibility tiles multiplied by
  precomputed 0/1 masks (DVE); attnT += v.T @ w (PE, accumulating);
  wsum += w (DVE/Pool alternating); denom = ones.T @ wsum (single 512-row
  matmul per head/tile); attnT *= broadcast(1/(denom + exp(sink))).
o_proj in transposed layout: outT[oc, tok] += woT.T @ attnT, written
  bf16 to a packed DRAM tensor; host sums partials and transposes (free).

Softmax uses a constant (zero) max-shift: scores are bounded (|s| < ~10)
far below fp32/bf16 exp overflow, and softmax is shift-invariant; the
sink logit enters the denominator as exp(sink).
"""
import os
import numpy as np
import ml_dtypes

import concourse.bass as bass
import concourse.mybir as mybir
import concourse.tile as tile
from concourse import bacc
from concourse.bass_utils import run_bass_kernel_spmd
from concourse.masks import make_identity
from contextlib import ExitStack

F32 = mybir.dt.float32
F32R = mybir.dt.float32r
BF16 = mybir.dt.bfloat16

S = 2048
HID = 4096
NQ = 32
NKV = 8
D = 128
WINDOW = 1024
THETA = 1e6
CORES = 8
QH = NQ // CORES          # 4 q heads per core
DQ = QH * D               # 512
NT = S // 512             # 4 token tiles of 512
KS = S // 128             # 16 key subtiles of 128

# q/k stored bf16 post-RoPE: the walrus verifier requires matmul operand
# dtypes to match when either is f32/f32r, and bf16 stationaries load 2x
# faster into the PE array.
KT_DT = BF16

last_results = None       # set by kernel(); test.py reads exec_time_ns


def _schedule(positions):
    """Static attention schedule from the actual positions array.

    Returns (masks_np [128, P*512] bf16, sched[qt] = list of (ks, pidx))
    where pidx is None for fully-visible key subtiles.
    """
    pos = np.asarray(positions).astype(np.int64)
    vis = (pos[None, :] <= pos[:, None]) & (pos[:, None] - pos[None, :] < WINDOW)
    patterns = {}
    plist = []
    sched = []
    for qt in range(NT):
        row = []
        for ks in range(KS):
            sub = vis[qt * 512:(qt + 1) * 512, ks * 128:(ks + 1) * 128]
            if not sub.any():
                continue
            if sub.all():
                row.append((ks, None))
            else:
                pat = np.ascontiguousarray(sub.T).astype(np.float32)  # [128 k, 512 q]
                key = pat.tobytes()
                if key not in patterns:
                    patterns[key] = len(plist)
                    plist.append(pat)
                row.append((ks, patterns[key]))
        sched.append(row)
    if not plist:
        plist = [np.ones((128, 512), np.float32)]
    masks = np.concatenate(plist, axis=1).astype(ml_dtypes.bfloat16)  # [128, P*512]
    return masks, sched, len(plist)


def _build(sched, n_patterns):
    nc = bacc.Bacc("TRN2", target_bir_lowering=False)

    # Host-packed inputs: every load is a contiguous [128, n] slab.
    HB = nc.dram_tensor("hb", [128, NT * 4 * 8 * 512], BF16, kind="ExternalInput")
    WKV = nc.dram_tensor("wkv", [128, 2 * 32 * 128], BF16, kind="ExternalInput")
    WQa = nc.dram_tensor("wqa", [128, 32 * 256], BF16, kind="ExternalInput")
    WQb = nc.dram_tensor("wqb", [128, 32 * 256], BF16, kind="ExternalInput")
    WOT = nc.dram_tensor("wot", [128, QH * 32 * 128], BF16, kind="ExternalInput")
    Cos = nc.dram_tensor("cos", [128, S], F32, kind="ExternalInput")
    Sin = nc.dram_tensor("sin", [128, S], F32, kind="ExternalInput")
    Mk = nc.dram_tensor("mk", [128, n_patterns * 512], BF16, kind="ExternalInput")
    One = nc.dram_tensor("one", [128, 1], BF16, kind="ExternalInput")
    Esk = nc.dram_tensor("esk", [1, QH], F32, kind="ExternalInput")
    # packed partial output: cols = qt*16384 + o*512 + t  (o = hid/128 chunk)
    OutP = nc.dram_tensor("outp", [128, NT * 32 * 512], BF16, kind="ExternalOutput")

    Exp = mybir.ActivationFunctionType.Exp

    with tile.TileContext(nc) as tc, ExitStack() as top:
        persist = top.enter_context(tc.tile_pool(name="persist", bufs=1))
        ones = persist.tile([128, 1], BF16)
        nc.sync.dma_start(ones[:], One[:])
        esk = persist.tile([1, QH], F32)
        nc.sync.dma_start(esk[:], Esk[:])
        cos_sb = persist.tile([128, S], F32)
        sin_sb = persist.tile([128, S], F32)
        mk_sb = persist.tile([128, n_patterns * 512], BF16)
        wot_sb = persist.tile([128, QH * 32 * 128], BF16)
        wq_sb = [persist.tile([128, 32 * 256], BF16, tag=f"wq{i}", name=f"wq{i}")
                 for i in range(2)]
        wkv_sb = persist.tile([128, 2 * 32 * 128], BF16)
        qT = [[persist.tile([128, 512], BF16, tag=f"qT{m}_{n}", name=f"qT{m}_{n}")
               for n in range(NT)] for m in range(QH)]
        kT = [persist.tile([128, 512], KT_DT, tag=f"kT{n}", name=f"kT{n}") for n in range(NT)]
        v_sb = [persist.tile([128, 512], BF16, tag=f"v{n}", name=f"v{n}") for n in range(NT)]

        def rope(dst, ps, n):
            co = cos_sb[:, n * 512:(n + 1) * 512]
            si = sin_sb[:, n * 512:(n + 1) * 512]
            t2 = rtmp.tile([128, 512], F32, tag="t2", name="t2")
            nc.vector.tensor_mul(t2[0:64, :], ps[64:128, :], si[0:64, :])
            nc.vector.tensor_mul(t2[64:128, :], ps[0:64, :], si[64:128, :])
            tc_ = rtmp.tile([128, 512], F32, tag="tc", name="tc")
            nc.vector.tensor_mul(tc_[:], ps[:], co)
            nc.vector.tensor_add(dst, tc_[:], t2[:])

        # ------- Slots: proj(n) [kv + q passes] + attn(n-1) + o_proj(n-2)
        with ExitStack() as pq:
            hqp = pq.enter_context(tc.tile_pool(name="hqp", bufs=1))
            rtmp = pq.enter_context(tc.tile_pool(name="rtmpq", bufs=2))
            vtp = pq.enter_context(tc.tile_pool(name="vtp", bufs=2))
            wpool = pq.enter_context(tc.tile_pool(name="wpool", bufs=7))
            wspool = pq.enter_context(tc.tile_pool(name="wspool", bufs=2))
            dpool = pq.enter_context(tc.tile_pool(name="dpool", bufs=2))
            apool = pq.enter_context(tc.tile_pool(name="apool", bufs=1))
            obp = pq.enter_context(tc.tile_pool(name="obp", bufs=2))
            ps_q = pq.enter_context(tc.tile_pool(name="ps_q", bufs=1, space="PSUM"))
            ps_s = pq.enter_context(tc.tile_pool(name="ps_s", bufs=2, space="PSUM"))
            ps_a = pq.enter_context(tc.tile_pool(name="ps_a", bufs=2, space="PSUM"))
            ps_o = pq.enter_context(tc.tile_pool(name="ps_o", bufs=2, space="PSUM"))

            attnT = [[None] * QH for _ in range(NT)]

            # Slot 0 is DMA-bound and a single queue tops out well below the
            # HBM rate, so its loads are spread across three issue queues in
            # first-use order: sync carries h, scalar carries wkv + cos/sin,
            # gpsimd carries wq + mk. wot (first use: slot 2) is enqueued
            # behind the first attention broadcast on gpsimd so its transfer
            # cannot steal bandwidth from the slot-0 critical path.
            for c in range(4):
                nc.gpsimd.dma_start(wkv_sb[:, c * 1024:(c + 1) * 1024],
                                    WKV[:, c * 1024:(c + 1) * 1024])
                nc.gpsimd.dma_start(wkv_sb[:, 4096 + c * 1024:4096 + (c + 1) * 1024],
                                    WKV[:, 4096 + c * 1024:4096 + (c + 1) * 1024])
            nc.gpsimd.dma_start(wq_sb[0][:], WQa[:])
            nc.gpsimd.dma_start(wq_sb[1][:], WQb[:])
            nc.gpsimd.dma_start(mk_sb[:], Mk[:])

            def proj_stream(n):
                hqs = []
                for q in range(4):
                    hq = hqp.tile([128, 4096], BF16, tag=f"hq{q}")
                    nc.sync.dma_start(hq[:], HB[:, (n * 4 + q) * 4096:(n * 4 + q + 1) * 4096])
                    if n == 0:
                        if q == 3:
                            nc.sync.dma_start(cos_sb[:, 0:512], Cos[:, 0:512])
                            nc.sync.dma_start(sin_sb[:, 0:512], Sin[:, 0:512])
                    elif q == 0 and n == 1:
                        nc.sync.dma_start(cos_sb[:, 512:1536], Cos[:, 512:1536])
                        nc.sync.dma_start(sin_sb[:, 512:1536], Sin[:, 512:1536])
                    elif q == 0 and n == 2:
                        nc.sync.dma_start(cos_sb[:, 1536:], Cos[:, 1536:])
                        nc.sync.dma_start(sin_sb[:, 1536:], Sin[:, 1536:])
                    hqs.append(hq)
                    yield
                # pass 1: k and v projections
                kvps = [ps_q.tile([128, 512], F32, tag=f"proj{j}", name=f"proj{j}")
                        for j in range(2)]
                for q in range(4):
                    for m in range(2):
                        for half in range(2):
                            for kk in range(half * 4, half * 4 + 4):
                                nc.tensor.matmul(
                                    kvps[m][:],
                                    wkv_sb[:, m * 4096 + (q * 8 + kk) * 128:
                                           m * 4096 + (q * 8 + kk + 1) * 128],
                                    hqs[q][:, kk * 512:(kk + 1) * 512],
                                    start=(q == 0 and kk == 0), stop=(q == 3 and kk == 7))
                            yield
                rope(kT[n][:], kvps[0][:], n)
                yield
                vt = vtp.tile([128, 512], BF16, tag="vt", name=f"vt{n}")
                nc.scalar.copy(vt[:], kvps[1][:])
                for t in range(4):
                    nc.sync.dma_start_transpose(v_sb[n][:, t * 128:(t + 1) * 128],
                                                vt[:, t * 128:(t + 1) * 128])
                yield
                # passes 2/3: q heads in pairs, reusing the same two banks
                for mp in range(2):
                    qps = [ps_q.tile([128, 512], F32, tag=f"proj{j}", name=f"proj{j}")
                           for j in range(2)]
                    for q in range(4):
                        for j in range(2):
                            for half in range(2):
                                for kk in range(half * 4, half * 4 + 4):
                                    nc.tensor.matmul(
                                        qps[j][:],
                                        wq_sb[mp][:, (q * 8 + kk) * 256 + j * 128:
                                                (q * 8 + kk) * 256 + (j + 1) * 128],
                                        hqs[q][:, kk * 512:(kk + 1) * 512],
                                        start=(q == 0 and kk == 0),
                                        stop=(q == 3 and kk == 7))
                                yield
                    for j in range(2):
                        rope(qT[mp * 2 + j][n][:], qps[j][:], n)
                        yield

            def b_stream(qt):
                row = sched[qt]
                last = len(row) - 1

                def a_mm(a_ps, i, ks, w):
                    nc.tensor.matmul(
                        a_ps[:], v_sb[ks // 4][:, (ks % 4) * 128:(ks % 4 + 1) * 128],
                        w[:], start=(i == 0), stop=(i == last))

                for hd in range(QH):
                    a_ps = ps_a.tile([128, 512], F32, tag="a")
                    wsum = wspool.tile([128, 512], BF16, tag="ws")
                    pend = None  # (i, ks, w): AV matmul deferred one chunk
                    for i, (ks, pidx) in enumerate(row):
                        s_ps = ps_s.tile([128, 512], F32, tag="s")
                        nc.tensor.matmul(
                            s_ps[:], kT[ks // 4][:, (ks % 4) * 128:(ks % 4 + 1) * 128],
                            qT[hd][qt][:], start=True, stop=True)
                        if pend is not None:
                            a_mm(a_ps, *pend)
                        w = wpool.tile([128, 512], BF16, tag="w")
                        nc.scalar.activation(w[:], s_ps[:], Exp)
                        if pidx is not None:
                            nc.vector.tensor_mul(
                                w[:], w[:], mk_sb[:, pidx * 512:(pidx + 1) * 512])
                        if i == 0:
                            nc.vector.tensor_copy(wsum[:], w[:])
                        else:
                            nc.vector.tensor_add(wsum[:], wsum[:], w[:])
                        pend = (i, ks, w)
                        yield
                    a_mm(a_ps, *pend)
                    d_ps = ps_s.tile([128, 512], F32, tag="s")
                    nc.tensor.matmul(d_ps[0:1, :], ones[:], wsum[:], start=True, stop=True)
                    den = dpool.tile([1, 512], F32, tag="den")
                    nc.vector.tensor_scalar_add(den[:], d_ps[0:1, :], esk[0:1, hd:hd + 1])
                    rec = dpool.tile([1, 512], F32, tag="rec")
                    nc.vector.reciprocal_approx_fast(rec[:], den[:])
                    rbc = dpool.tile([128, 512], F32, tag="rbc")
                    nc.gpsimd.partition_broadcast(rbc[:], rec[:])
                    if qt == 0 and hd == 0:
                        # deferred bulk prefetch: queued after the broadcast
                        # above, so it starts once slot 1 is underway
                        nc.gpsimd.dma_start(wot_sb[:], WOT[:])
                    at = apool.tile([128, 512], BF16, tag=f"at{hd}_{qt % 2}")
                    nc.vector.tensor_mul(at[:], a_ps[:], rbc[:])
                    attnT[qt][hd] = at
                    yield

            def c_stream(qt):
                # the last tile runs with no other stream to hide the PSUM
                # drain, so borrow the idle proj banks for 4-deep pipelining
                deep = qt == NT - 1
                for og in range(8):
                    ob = obp.tile([128, 2048], BF16, tag="ob")
                    for c in range(4):
                        o = og * 4 + c
                        if deep and c % 2:
                            o_ps = ps_q.tile([128, 512], F32, tag=f"proj{og % 2}",
                                             name="o_ps")
                        else:
                            o_ps = ps_o.tile([128, 512], F32, tag="o")
                        for hd in range(QH):
                            nc.tensor.matmul(
                                o_ps[:],
                                wot_sb[:, (hd * 32 + o) * 128:(hd * 32 + o + 1) * 128],
                                attnT[qt][hd][:],
                                start=(hd == 0), stop=(hd == QH - 1))
                        if c % 2:
                            nc.scalar.copy(ob[:, c * 512:(c + 1) * 512], o_ps[:])
                        else:
                            nc.vector.tensor_copy(ob[:, c * 512:(c + 1) * 512], o_ps[:])
                        yield
                    nc.gpsimd.dma_start(
                        OutP[:, qt * 16384 + og * 2048:qt * 16384 + (og + 1) * 2048], ob[:])
                    yield
                    if not deep:
                        yield  # stretch to pace the longer b_stream rows

            for slot in range(NT + 2):
                streams = []
                if slot < NT:
                    streams.append(proj_stream(slot))
                if 0 <= slot - 1 < NT:
                    streams.append(b_stream(slot - 1))
                if 0 <= slot - 2 < NT:
                    streams.append(c_stream(slot - 2))
                while streams:
                    alive = []
                    for st in streams:
                        try:
                            next(st)
                            alive.append(st)
                        except StopIteration:
                            pass
                    streams = alive

    nc.compile()
    return nc


def kernel(hidden_states, positions, wq, wk, wv, wo, sink):
    global last_results
    h = np.asarray(hidden_states, np.float32)
    pos = np.asarray(positions)
    wq = np.asarray(wq, np.float32)
    wk = np.asarray(wk, np.float32)
    wv = np.asarray(wv, np.float32)
    wo = np.asarray(wo, np.float32)
    sink = np.asarray(sink, np.float32)

    masks, sched, n_pat = _schedule(pos)
    nc = _build(sched, n_pat)

    # h packed: [p, (n, q, kt, t)] = h[n*512+t, (q*8+kt)*128+p]
    hp = np.ascontiguousarray(
        h.reshape(NT, 512, 4, 8, 128).transpose(4, 0, 2, 3, 1).reshape(128, -1)
    ).astype(ml_dtypes.bfloat16)

    # RoPE tables (neox half-split), rows duplicated for both halves
    inv_freq = 1.0 / (THETA ** (np.arange(0, D, 2, dtype=np.float64) / D))
    freqs = pos.astype(np.float64)[:, None] * inv_freq[None, :]       # [S, 64]
    cos = np.cos(freqs).astype(np.float32).T                          # [64, S]
    sin = np.sin(freqs).astype(np.float32).T
    cos_full = np.ascontiguousarray(np.concatenate([cos, cos], axis=0))
    sin_sign = np.ascontiguousarray(np.concatenate([-sin, sin], axis=0))

    scale = np.float32(D ** -0.5)
    ones = np.ones((128, 1), np.float32)
    esink = np.exp(sink.astype(np.float64)).astype(np.float32)

    in_maps = []
    for c in range(CORES):
        wqc = (wq[:, c * DQ:(c + 1) * DQ] * scale)                    # [HID, 512]
        # [p, kt, m*128+i] = wqc[kt*128+p, :], split into head pairs
        wqr = wqc.reshape(32, 128, 512).transpose(1, 0, 2)
        wqa = np.ascontiguousarray(wqr[:, :, 0:256].reshape(128, -1)).astype(ml_dtypes.bfloat16)
        wqb = np.ascontiguousarray(wqr[:, :, 256:512].reshape(128, -1)).astype(ml_dtypes.bfloat16)
        wkc = wk[:, c * D:(c + 1) * D].reshape(32, 128, 128).transpose(1, 0, 2)
        wvc = wv[:, c * D:(c + 1) * D].reshape(32, 128, 128).transpose(1, 0, 2)
        wkvp = np.ascontiguousarray(
            np.concatenate([wkc.reshape(128, -1), wvc.reshape(128, -1)], axis=1)
        ).astype(ml_dtypes.bfloat16)
        woc = wo[c * DQ:(c + 1) * DQ, :]                              # [512, HID]
        # [p, hd, o, i] = woc[hd*128+p, o*128+i]
        wotp = np.ascontiguousarray(
            woc.reshape(QH, 128, 32, 128).transpose(1, 0, 2, 3).reshape(128, -1)
        ).astype(ml_dtypes.bfloat16)
        in_maps.append({
            "hb": hp,
            "wqa": wqa,
            "wqb": wqb,
            "wkv": wkvp,
            "wot": wotp,
            "cos": cos_full,
            "sin": sin_sign,
            "mk": masks,
            "one": ones.astype(ml_dtypes.bfloat16),
            "esk": np.ascontiguousarray(esink[None, c * QH:(c + 1) * QH]),
        })

    trace = bool(int(os.environ.get("KERNEL_TRACE", "0")))
    res = run_bass_kernel_spmd(nc, in_maps, core_ids=list(range(CORES)), trace=trace)
    last_results = res
    acc = np.zeros((128, NT * 32 * 512), np.float64)
    for r in res.results:
        acc += r["outp"].astype(np.float64)
    # [p, qt, o, t] -> out[qt*512+t, o*128+p]
    out = acc.reshape(128, NT, 32, 512).transpose(1, 3, 2, 0).reshape(S, HID)
    return out.astype(np.float32)
